# revision 1
# baseline (speedup 1.0000x reference)
"""ChannelAttention Trainium2 Bass kernel.

Data-parallel over batch: 8 batches -> 8 NeuronCores, zero communication.

Key algebra: q,k are never materialized.  With G = x^T x  [C, C]:
  gram_h   = q_h^T k_h = Wq_h^T G Wk_h
  ||q_d||^2 = diag(Wq_h^T G Wq_h)  (same for k)
so pass 1 only accumulates G (upper triangle, symmetric) from streamed
token chunks.  The finalize runs on [768 x 1536] matrices:
  A2 = G @ [Wq | Wk]; gram_h = Wq_h^T A2k_h; sumsq = colsum(W2 * A2)
then softmax and the fused output matrix
  Wbig = sum_h Wv_h @ attn_h^T @ Wproj_h          [C, C]
Pass 2 computes y^T = Wbig^T @ x^T + b in bf16, streaming a
host-supplied bf16 x^T (no on-device transposes); host transposes back.

Weight/output DMAs ride the scalar HWDGE ring so the x streams on the
sync ring are never queued behind them.
"""

import sys

if "/opt/trn_rl_repo" not in sys.path:
    sys.path.insert(0, "/opt/trn_rl_repo")

import numpy as np

N, C, H, HD = 4096, 768, 8, 96
C2 = 2 * C
NC3 = 3 * C
EPS = 1e-12
P = 128
CB = C // P            # 6 channel blocks
NCH = N // P           # 32 token chunks

# upper-triangle block packing: block (r, c), r <= c, index b -> bank b//4,
# column offset (b%4)*128 inside PSUM tiles of [128, 512]
_STARTS = [0, 6, 11, 15, 18, 20]
# per row: list of (bank, offset, c0, ncols) matmul runs covering cols c0..
_G_RUNS = {
    0: [(0, 0, 0, 512), (1, 0, 4, 256)],
    1: [(1, 256, 1, 256), (2, 0, 3, 384)],
    2: [(2, 384, 2, 128), (3, 0, 3, 384)],
    3: [(3, 384, 3, 128), (4, 0, 4, 256)],
    4: [(4, 256, 4, 256)],
    5: [(5, 0, 5, 128)],
}

_CACHE = {}


def _blk(b):
    return b // 4, (b % 4) * P


def _build(dbg=False):
    import concourse.bacc as bacc
    import concourse.tile as tile
    import concourse.mybir as mybir
    from concourse.masks import make_identity
    from contextlib import ExitStack

    F32 = mybir.dt.float32
    F32R = mybir.dt.float32r
    BF16 = mybir.dt.bfloat16

    def R(ap):
        return ap.bitcast(F32R)

    nc = bacc.Bacc("TRN2", target_bir_lowering=False, debug=False, num_devices=8)
    x = nc.dram_tensor("x", [N, C], BF16, kind="ExternalInput")
    xt = nc.dram_tensor("xt", [C, N], BF16, kind="ExternalInput")
    w2 = nc.dram_tensor("w2", [C, C2], F32, kind="ExternalInput")
    wvt = nc.dram_tensor("wvt", [HD, H, C], F32, kind="ExternalInput")
    wpe = nc.dram_tensor("wpe", [HD, H, C], F32, kind="ExternalInput")
    temp = nc.dram_tensor("temp", [H], F32, kind="ExternalInput")
    biasE = nc.dram_tensor("biasE", [P, CB], F32, kind="ExternalInput")
    yt = nc.dram_tensor("yt", [C, N], BF16, kind="ExternalOutput")
    if dbg:
        dbg_g = nc.dram_tensor("dbg_g", [P, CB, C], F32, kind="ExternalOutput")
        dbg_ss = nc.dram_tensor("dbg_ss", [HD, 2 * H], F32, kind="ExternalOutput")
        dbg_s = nc.dram_tensor("dbg_s", [HD, 2 * H], F32, kind="ExternalOutput")
        dbg_at = nc.dram_tensor("dbg_at", [HD, H, HD], F32, kind="ExternalOutput")
        dbg_wb = nc.dram_tensor("dbg_wb", [P, CB, C], BF16, kind="ExternalOutput")
        dbg_t1 = nc.dram_tensor("dbg_t1", [HD, H, C], F32, kind="ExternalOutput")
        dbg_pp = nc.dram_tensor("dbg_pp", [P, C2], F32, kind="ExternalOutput")

    with tile.TileContext(nc) as tc, ExitStack() as ctx:
        singles = ctx.enter_context(tc.tile_pool(name="singles", bufs=1))
        ident_f = singles.tile([P, P], F32)
        ident_r = singles.tile([P, P], F32R)
        ones_f = singles.tile([P, HD], F32)
        ones1 = singles.tile([P, 1], F32R)
        temp_sb = singles.tile([HD, H], F32)
        bias_sb = singles.tile([P, CB], F32)
        s_sb = singles.tile([HD, 2 * H], F32)
        sumsq_sb = singles.tile([HD, 2 * H], F32)
        atsb = singles.tile([HD, H, HD], F32R)
        wbig_sb = singles.tile([P, CB, C], BF16)

        make_identity(nc, ident_f)
        nc.vector.tensor_copy(out=ident_r, in_=ident_f)
        nc.vector.memset(ones_f, 1.0)
        nc.vector.tensor_copy(out=ones1, in_=ones_f[:, 0:1])
        nc.scalar.dma_start(out=temp_sb, in_=temp[None, :].to_broadcast([HD, H]))
        nc.scalar.dma_start(out=bias_sb, in_=biasE[:, :])

        # finalize weights stream on the scalar ring during pass 1
        wvt_sb = singles.tile([HD, H, C], F32R)
        wpe_sb = singles.tile([HD, H, C], F32R)
        c_ctx = ExitStack()
        cpool = c_ctx.enter_context(tc.tile_pool(name="xtcp", bufs=3))
        ypool = c_ctx.enter_context(tc.tile_pool(name="ysbp", bufs=4))

        w2_ctx = ExitStack()
        w2_pool = w2_ctx.enter_context(tc.tile_pool(name="w2p", bufs=1))
        w2_sb = w2_pool.tile([P, CB, C2], F32R)
        nc.scalar.dma_start(
            out=w2_sb, in_=R(w2.rearrange("(cb p) j -> p cb j", p=P))
        )
        nc.scalar.dma_start(out=wvt_sb, in_=R(wvt[:, :, :]))
        nc.scalar.dma_start(out=wpe_sb, in_=R(wpe[:, :, :]))

        # ---------------- pass 1: G = x^T x (upper triangle) ----------------
        gram_ctx = ExitStack()
        gram_pool = gram_ctx.enter_context(
            tc.tile_pool(name="gps", bufs=1, space="PSUM")
        )
        gtile = [
            gram_pool.tile([P, 512], F32, tag=f"g{i}", name=f"g{i}")
            for i in range(6)
        ]

        with tc.tile_pool(name="p1", bufs=6) as p1pool:
            for i in range(NCH):
                xc = p1pool.tile([P, C], BF16, tag="xc")
                nc.sync.dma_start(out=xc, in_=x[i * P : (i + 1) * P, :])
                for r in range(CB):
                    lh = xc[:, r * P : (r + 1) * P]
                    for (bank, off, c0, ncols) in _G_RUNS[r]:
                        nc.tensor.matmul(
                            gtile[bank][:, off : off + ncols],
                            lhsT=lh,
                            rhs=xc[:, c0 * P : c0 * P + ncols],
                            start=(i == 0 and off == 0),
                            stop=(i == NCH - 1),
                            skip_group_check=True,
                        )

        # ---------------- finalize ----------------
        fs_ctx = ExitStack()
        fsb = fs_ctx.enter_context(tc.tile_pool(name="fsb", bufs=2))
        fsb2 = fs_ctx.enter_context(tc.tile_pool(name="fsb2", bufs=1))
        a2pool_sb = fs_ctx.enter_context(tc.tile_pool(name="a2sb", bufs=2))
        pp = fs_ctx.enter_context(tc.tile_pool(name="ppp", bufs=1)).tile(
            [P, C2], F32R
        )

        gsb_ctx = ExitStack()
        gsb_pool = gsb_ctx.enter_context(tc.tile_pool(name="gsbp", bufs=1))
        gsb = gsb_pool.tile([P, CB, C], F32R)

        # PSUM -> SBUF upper blocks (split DVE/ACT), mirror via PE matmul
        for r in range(CB):
            for c in range(r, CB):
                bank, off = _blk(_STARTS[r] + c - r)
                if (r + c) % 2 == 0:
                    nc.vector.tensor_copy(
                        out=gsb[:, r, c * P : (c + 1) * P],
                        in_=gtile[bank][:, off : off + P],
                    )
                else:
                    nc.scalar.copy(
                        out=gsb[:, r, c * P : (c + 1) * P],
                        in_=gtile[bank][:, off : off + P],
                    )
        gram_ctx.close()

        with tc.tile_pool(name="tpps", bufs=2, space="PSUM") as tppool:
            for r in range(CB):
                for c in range(r + 1, CB):
                    tp = tppool.tile([P, P], F32, tag="tp")
                    nc.tensor.matmul(
                        tp,
                        lhsT=gsb[:, r, c * P : (c + 1) * P],
                        rhs=ident_r,
                        start=True,
                        stop=True,
                    )
                    nc.vector.tensor_copy(
                        out=gsb[:, c, r * P : (r + 1) * P], in_=tp
                    )

        if dbg:
            nc.scalar.dma_start(out=dbg_g[:, :, :], in_=gsb.bitcast(F32))

        # A2 = G @ [Wq | Wk] row-by-row, with per-row gram matmuls and
        # sum-of-squares products interleaved so PE stays dense.
        at_ctx = ExitStack()
        atpool = at_ctx.enter_context(
            tc.tile_pool(name="atps", bufs=1, space="PSUM")
        )
        at1 = atpool.tile([HD, 5 * HD], F32, tag="at1", name="at1")
        at2 = atpool.tile([HD, 3 * HD], F32, tag="at2", name="at2")

        with tc.tile_pool(name="a2ps", bufs=2, space="PSUM") as a2pool:
            for r in range(CB):
                a2p = a2pool.tile([P, C2], F32, tag="a2p")
                for cb in range(CB):
                    lh = gsb[:, cb, r * P : (r + 1) * P]
                    for js in range(3):
                        nc.tensor.matmul(
                            a2p[:, js * 512 : (js + 1) * 512],
                            lhsT=lh,
                            rhs=w2_sb[:, cb, js * 512 : (js + 1) * 512],
                            start=(cb == 0),
                            stop=(cb == CB - 1),
                            skip_group_check=True,
                        )
                a2row = a2pool_sb.tile([P, C2], F32R, tag="a2row")
                nc.vector.tensor_copy(out=a2row, in_=a2p)
                # gram contributions of this row
                for h in range(H):
                    bank = at1 if h < 5 else at2
                    co = HD * h if h < 5 else HD * (h - 5)
                    nc.tensor.matmul(
                        bank[:, co : co + HD],
                        lhsT=w2_sb[:, r, h * HD : (h + 1) * HD],
                        rhs=a2row[:, C + h * HD : C + (h + 1) * HD],
                        start=(r == 0 and h in (0, 5)),
                        stop=(r == CB - 1),
                        skip_group_check=True,
                    )
                # sum-of-squares partial products
                if r == 0:
                    nc.vector.tensor_tensor(
                        out=pp, in0=w2_sb[:, 0, :], in1=a2row,
                        op=mybir.AluOpType.mult,
                    )
                else:
                    pt = fsb.tile([P, C2], F32R, tag="pt")
                    nc.vector.tensor_tensor(
                        out=pt, in0=w2_sb[:, r, :], in1=a2row,
                        op=mybir.AluOpType.mult,
                    )
                    nc.vector.tensor_tensor(
                        out=pp, in0=pp, in1=pt, op=mybir.AluOpType.add
                    )
        gsb_ctx.close()

        with tc.tile_pool(name="ssps", bufs=1, space="PSUM") as sspool:
            ssp = [
                sspool.tile([1, 512], F32, tag=f"ss{j}", name=f"ss{j}")
                for j in range(3)
            ]
            sqp = sspool.tile([HD, 2 * H], F32, tag="sqp", name="sqp")
            for js in range(3):
                nc.tensor.matmul(
                    ssp[js],
                    lhsT=ones1,
                    rhs=pp[:, js * 512 : (js + 1) * 512],
                    start=True,
                    stop=True,
                )
            ssrow = fsb2.tile([1, C2], F32, tag="ssrow")
            for js in range(3):
                nc.vector.tensor_copy(
                    out=ssrow[:, js * 512 : (js + 1) * 512], in_=ssp[js]
                )
            # row -> columns [96, 16] via K=1 fp32 matmuls
            for t in range(2):
                for h in range(H):
                    j = t * H + h
                    f0 = t * C + h * HD
                    nc.tensor.matmul(
                        sqp[:, j : j + 1],
                        lhsT=ssrow[0:1, f0 : f0 + HD],
                        rhs=ones_f[0:1, 0:1],
                        start=(j == 0),
                        stop=(j == 2 * H - 1),
                        skip_group_check=True,
                    )
            nc.vector.tensor_copy(out=sumsq_sb, in_=sqp)

            # s = 1/max(sqrt(ss), eps); temperature folded into s_q
            nc.scalar.sqrt(out=s_sb, in_=sumsq_sb)
            nc.vector.tensor_scalar_max(s_sb, s_sb, EPS)
            nc.vector.reciprocal(out=s_sb, in_=s_sb)
            nc.vector.tensor_tensor(
                out=s_sb[:, 0:H],
                in0=s_sb[:, 0:H],
                in1=temp_sb,
                op=mybir.AluOpType.mult,
            )

        # combined scale [d,h,e] = s_q[d,h] * s_k[h,e] via ones96^T @ diag
        if True:
            with tc.tile_pool(name="skps", bufs=1, space="PSUM") as skpool:
                ones96 = fsb2.tile([HD, HD], F32R, tag="ones96")
                nc.vector.tensor_copy(out=ones96, in_=ones_f[0:HD, :])
                diag_all = fsb2.tile([HD, H, HD], F32R, tag="diag_all")
                nc.vector.tensor_tensor(
                    out=diag_all,
                    in0=ident_r[0:HD, None, 0:HD].to_broadcast([HD, H, HD]),
                    in1=s_sb[:, H : 2 * H, None].to_broadcast([HD, H, HD]),
                    op=mybir.AluOpType.mult,
                )
                skp = skpool.tile([HD, H * HD], F32, tag="skp")
                df = R(diag_all).rearrange("p h e -> p (h e)")
                nc.tensor.matmul(
                    skp[:, 0:512], lhsT=ones96, rhs=df[:, 0:512],
                    start=True, stop=True,
                )
                nc.tensor.matmul(
                    skp[:, 512:768], lhsT=ones96, rhs=df[:, 512:768],
                    start=True, stop=True,
                )
                skrep = fsb2.tile([HD, H, HD], F32, tag="skrep")
                nc.vector.tensor_copy(
                    out=skrep.rearrange("p h e -> p (h e)"), in_=skp
                )
                nc.vector.tensor_tensor(
                    out=skrep,
                    in0=skrep,
                    in1=s_sb[:, 0:H, None].to_broadcast([HD, H, HD]),
                    op=mybir.AluOpType.mult,
                )

            # softmax per head-group (no max subtraction: |attn| <= temp),
            # T1_h = attn_h^T @ Wproj_h follows each group on PE
            t1_ctx = ExitStack()
            t1_pool = t1_ctx.enter_context(tc.tile_pool(name="t1p", bufs=1))
            t1_sb = t1_pool.tile([HD, H, C], F32R)
            with tc.tile_pool(name="t1ps", bufs=2, space="PSUM") as t1ps:
                for g, (h0, nh) in enumerate(((0, 5), (5, 3))):
                    bank = at1 if g == 0 else at2
                    ga = atsb[:, h0 : h0 + nh, :]
                    nc.vector.tensor_copy(
                        out=ga.rearrange("p h e -> p (h e)"), in_=bank
                    )
                    nc.vector.tensor_tensor(
                        out=ga, in0=ga, in1=skrep[:, h0 : h0 + nh, :],
                        op=mybir.AluOpType.mult,
                    )
                    nc.scalar.activation(
                        out=ga, in_=ga,
                        func=mybir.ActivationFunctionType.Exp,
                        bias=0.0, scale=1.0,
                    )
                    rsum = fsb.tile([HD, H], F32, tag="rsum")
                    nc.vector.tensor_reduce(
                        out=rsum[:, 0:nh], in_=ga, axis=mybir.AxisListType.X,
                        op=mybir.AluOpType.add,
                    )
                    nc.vector.reciprocal(out=rsum[:, 0:nh], in_=rsum[:, 0:nh])
                    nc.vector.tensor_tensor(
                        out=ga, in0=ga,
                        in1=rsum[:, 0:nh, None].to_broadcast([HD, nh, HD]),
                        op=mybir.AluOpType.mult,
                    )
                    for h in range(h0, h0 + nh):
                        t1p = t1ps.tile([HD, C], F32, tag="t1p")
                        lh = atsb[:, h, :]
                        nc.tensor.matmul(
                            t1p[:, 0:512], lhsT=lh, rhs=wpe_sb[:, h, 0:512],
                            start=True, stop=True,
                        )
                        nc.tensor.matmul(
                            t1p[:, 512:C], lhsT=lh, rhs=wpe_sb[:, h, 512:C],
                            start=True, stop=True,
                        )
                        nc.vector.tensor_copy(out=t1_sb[:, h, :], in_=t1p)
        if dbg:
            nc.scalar.dma_start(out=dbg_at[:, :, :], in_=atsb.bitcast(F32))
            nc.scalar.dma_start(out=dbg_t1[:, :, :], in_=t1_sb.bitcast(F32))
        at_ctx.close()

        # Wbig = sum_h Wv_h @ T1_h
        with tc.tile_pool(name="wbps", bufs=2, space="PSUM") as wbps:
            for m in range(CB):
                wbp = wbps.tile([P, C], F32, tag="wbp")
                for h in range(H):
                    lh = wvt_sb[:, h, m * P : (m + 1) * P]
                    nc.tensor.matmul(
                        wbp[:, 0:512], lhsT=lh, rhs=t1_sb[:, h, 0:512],
                        start=(h == 0), stop=(h == H - 1),
                        skip_group_check=True,
                    )
                    nc.tensor.matmul(
                        wbp[:, 512:C], lhsT=lh, rhs=t1_sb[:, h, 512:C],
                        start=(h == 0), stop=(h == H - 1),
                        skip_group_check=True,
                    )
                nc.vector.tensor_copy(out=wbig_sb[:, m, :], in_=wbp)
        if dbg:
            nc.scalar.dma_start(out=dbg_wb[:, :, :], in_=wbig_sb)
        t1_ctx.close()
        fs_ctx.close()
        w2_ctx.close()

        # ---------------- pass 2: y^T = Wbig^T @ x^T + b (bf16) -------------
        with tc.tile_pool(name="yps", bufs=8, space="PSUM") as yps:
            for n in range(8):
                nsl = slice(n * 512, (n + 1) * 512)
                xtc = cpool.tile([P, CB, 512], BF16, tag="xtc")
                nc.sync.dma_start(
                    out=xtc,
                    in_=xt[:, nsl].rearrange("(cb p) m -> p cb m", p=P),
                )
                for co in range(CB):
                    ypb = yps.tile([P, 512], F32, tag="ypb")
                    for ci in range(CB):
                        nc.tensor.matmul(
                            ypb,
                            lhsT=wbig_sb[:, ci, co * P : (co + 1) * P],
                            rhs=xtc[:, ci, :],
                            start=(ci == 0),
                            stop=(ci == CB - 1),
                        )
                    ysb = ypool.tile([P, 512], BF16, tag="ysb")
                    nc.vector.tensor_tensor(
                        out=ysb,
                        in0=ypb,
                        in1=bias_sb[:, co : co + 1].to_broadcast([P, 512]),
                        op=mybir.AluOpType.add,
                    )
                    nc.scalar.dma_start(
                        out=yt[co * P : (co + 1) * P, nsl], in_=ysb
                    )
        c_ctx.close()

    nc.compile()
    return nc


def prep_inputs(x, Wqkv, temperature, Wproj, bproj):
    import ml_dtypes

    B = x.shape[0]
    wqkv = np.asarray(Wqkv, dtype=np.float32)
    w2 = np.ascontiguousarray(wqkv[:, :C2])
    wvt = np.ascontiguousarray(
        wqkv[:, C2:].T.reshape(H, HD, C).transpose(1, 0, 2)
    )
    wpe = np.ascontiguousarray(
        np.asarray(Wproj, dtype=np.float32).reshape(H, HD, C).transpose(1, 0, 2)
    )
    temp = np.ascontiguousarray(np.asarray(temperature, dtype=np.float32).reshape(H))
    biasE = np.ascontiguousarray(
        np.asarray(bproj, dtype=np.float32).reshape(CB, P).T
    )
    xf = np.asarray(x, dtype=np.float32)
    in_maps = [
        {
            "x": np.ascontiguousarray(xf[b].astype(ml_dtypes.bfloat16)),
            "xt": np.ascontiguousarray(xf[b].T.astype(ml_dtypes.bfloat16)),
            "w2": w2,
            "wvt": wvt,
            "wpe": wpe,
            "temp": temp,
            "biasE": biasE,
        }
        for b in range(B)
    ]
    return in_maps


def kernel(x, Wqkv, temperature, Wproj, bproj):
    from concourse.bass_utils import run_bass_kernel_spmd

    B = x.shape[0]
    key = "nc"
    if key not in _CACHE:
        _CACHE[key] = _build()
    nc = _CACHE[key]

    in_maps = prep_inputs(x, Wqkv, temperature, Wproj, bproj)
    res = run_bass_kernel_spmd(nc, in_maps, core_ids=list(range(B)))
    out = np.stack(
        [res.results[b]["yt"].T.astype(np.float32) for b in range(B)], axis=0
    )
    return np.ascontiguousarray(out)


if __name__ == "__main__":
    rng = np.random.default_rng(0)
    inputs = {
        "x": rng.standard_normal((8, N, C), dtype=np.float32),
        "Wqkv": (rng.standard_normal((C, NC3)) / np.sqrt(C)).astype(np.float32),
        "temperature": np.ones((H, 1, 1), dtype=np.float32),
        "Wproj": (rng.standard_normal((C, C)) / np.sqrt(C)).astype(np.float32),
        "bproj": (rng.standard_normal(C) * 0.01).astype(np.float32),
    }
    out = kernel(**inputs)
    print(out.shape, out.dtype)



# revision 5
# speedup vs baseline: 1.2315x; 1.2315x over previous
"""ChannelAttention Trainium2 Bass kernel (v2).

Data-parallel over batch: 8 batches -> 8 NeuronCores, zero communication.

Key algebra: q,k are never materialized.  With G = x^T x  [C, C]:
  gram_qk_h = Wq_h^T G Wk_h          (attention logits numerator)
  sumsq_q   = diag(Wq_h^T G Wq_h)    (token-dim L2 norms of q)
  sumsq_k   = diag(Wk_h^T G Wk_h)
Pass 1 accumulates G (upper triangle) from streamed token chunks.
Finalize (all fp16 matmul operands, fp32 PSUM):
  phase K: A2k = G @ Wk, then per-head grams Wq_h^T A2k_h (qk) and
           Wk_h^T A2k_h (kk; diag -> sumsq_k)
  phase Q: A2q = G @ Wq, per-head grams Wq_h^T A2q_h (qq; diag -> sumsq_q)
  softmax, T1_h = attn_h^T Wproj_h, Wbig = sum_h Wv_h @ T1_h   [C, C]
Pass 2: y^T = Wbig^T @ x^T + b from an SBUF-resident x^T (prefetched on
the sync ring during pass 1 / finalize).  Host transposes the output.

fp16 everywhere (not bf16): same PE throughput, 8x finer mantissa; all
values here are << fp16 max (|x|<6, |G|<5k, |A2|<1k).
"""

import sys

if "/opt/trn_rl_repo" not in sys.path:
    sys.path.insert(0, "/opt/trn_rl_repo")

import numpy as np

N, C, H, HD = 4096, 768, 8, 96
C2 = 2 * C
NC3 = 3 * C
EPS = 1e-12
P = 128
CB = C // P            # 6 channel blocks
NCH2 = N // 256        # 16 double token chunks

# upper-triangle block packing: block (r, c), r <= c, index b -> bank b//4,
# column offset (b%4)*128 inside PSUM tiles of [128, 512]
_STARTS = [0, 6, 11, 15, 18, 20]
# per row: list of (bank, offset, c0, ncols) matmul runs covering cols c0..
_G_RUNS = {
    0: [(0, 0, 0, 512), (1, 0, 4, 256)],
    1: [(1, 256, 1, 256), (2, 0, 3, 384)],
    2: [(2, 384, 2, 128), (3, 0, 3, 384)],
    3: [(3, 384, 3, 128), (4, 0, 4, 256)],
    4: [(4, 256, 4, 256)],
    5: [(5, 0, 5, 128)],
}

_CACHE = {}


def _blk(b):
    return b // 4, (b % 4) * P


def _build(dbg=False):
    import concourse.bacc as bacc
    import concourse.tile as tile
    import concourse.mybir as mybir
    from concourse.masks import make_identity
    from contextlib import ExitStack

    F32 = mybir.dt.float32
    F16 = mybir.dt.float16

    nc = bacc.Bacc("TRN2", target_bir_lowering=False, debug=False, num_devices=8)
    x = nc.dram_tensor("x", [N, C], F16, kind="ExternalInput")
    xt = nc.dram_tensor("xt", [C, N], F16, kind="ExternalInput")
    w2 = nc.dram_tensor("w2", [C, C2], F16, kind="ExternalInput")
    wvt = nc.dram_tensor("wvt", [HD, H, C], F16, kind="ExternalInput")
    wpe = nc.dram_tensor("wpe", [HD, H, C], F16, kind="ExternalInput")
    temp = nc.dram_tensor("temp", [H], F32, kind="ExternalInput")
    biasE = nc.dram_tensor("biasE", [P, CB], F32, kind="ExternalInput")
    yt = nc.dram_tensor("yt", [C, N], F16, kind="ExternalOutput")
    if dbg:
        dbg_g = nc.dram_tensor("dbg_g", [P, CB, C], F16, kind="ExternalOutput")
        dbg_a2 = nc.dram_tensor("dbg_a2", [P, CB, C2], F16, kind="ExternalOutput")
        dbg_ss = nc.dram_tensor("dbg_ss", [HD, 2 * H], F32, kind="ExternalOutput")
        dbg_at = nc.dram_tensor("dbg_at", [HD, H, HD], F16, kind="ExternalOutput")
        dbg_t1 = nc.dram_tensor("dbg_t1", [HD, H, C], F16, kind="ExternalOutput")
        dbg_wb = nc.dram_tensor("dbg_wb", [P, CB, C], F16, kind="ExternalOutput")

    with tile.TileContext(nc) as tc, ExitStack() as ctx:
        singles = ctx.enter_context(tc.tile_pool(name="singles", bufs=1))
        ident_f = singles.tile([P, P], F32)
        ident_h = singles.tile([P, P], F16)
        ones96 = singles.tile([HD, HD], F16)
        temp_sb = singles.tile([HD, H], F32)
        bias_sb = singles.tile([P, CB], F32)
        s_sb = singles.tile([HD, 2 * H], F32)
        sumsq_sb = singles.tile([HD, 2 * H], F32)
        atsb = singles.tile([HD, H, HD], F16)
        xt_sb = singles.tile([P, CB, N], F16)
        w2_sb = singles.tile([P, CB, C2], F16)
        wvt_sb = singles.tile([HD, H, C], F16)
        wpe_sb = singles.tile([HD, H, C], F16)
        gsb = singles.tile([P, CB, C], F16)
        a2sb = singles.tile([P, CB, C2], F16)
        t1_sb = singles.tile([HD, H, C], F16)
        wbig_sb = singles.tile([P, CB, C], F16)

        make_identity(nc, ident_f)
        nc.vector.tensor_copy(out=ident_h, in_=ident_f)
        nc.vector.memset(ones96, 1.0)
        nc.scalar.dma_start(out=temp_sb, in_=temp[None, :].to_broadcast([HD, H]))
        nc.scalar.dma_start(out=bias_sb, in_=biasE[:, :])
        # weights on the scalar HWDGE ring; x/xt stream on the sync ring
        nc.scalar.dma_start(
            out=w2_sb, in_=w2.rearrange("(cb p) j -> p cb j", p=P)
        )
        nc.scalar.dma_start(out=wvt_sb, in_=wvt[:, :, :])
        nc.scalar.dma_start(out=wpe_sb, in_=wpe[:, :, :])

        # ---------------- pass 1: G = x^T x (upper triangle) ----------------
        gram_ctx = ExitStack()
        gram_pool = gram_ctx.enter_context(
            tc.tile_pool(name="gps", bufs=1, space="PSUM")
        )
        gtile = [
            gram_pool.tile([P, 512], F32, tag=f"g{i}", name=f"g{i}")
            for i in range(6)
        ]

        with tc.tile_pool(name="p1", bufs=8) as p1pool:
            for i in range(NCH2):
                xc = p1pool.tile([P, 2, C], F16, tag="xc")
                nc.sync.dma_start(
                    out=xc,
                    in_=x[i * 256 : (i + 1) * 256, :].rearrange(
                        "(two p) c -> p two c", p=P
                    ),
                )
                for t in range(2):
                    for r in range(CB):
                        lh = xc[:, t, r * P : (r + 1) * P]
                        for (bank, off, c0, ncols) in _G_RUNS[r]:
                            nc.tensor.matmul(
                                gtile[bank][:, off : off + ncols],
                                lhsT=lh,
                                rhs=xc[:, t, c0 * P : c0 * P + ncols],
                                start=(i == 0 and t == 0 and off == 0),
                                stop=(i == NCH2 - 1 and t == 1),
                                skip_group_check=True,
                            )

        # prefetch x^T fully into SBUF on the sync ring (behind the x chunks)
        for n in range(8):
            nsl = slice(n * 512, (n + 1) * 512)
            nc.sync.dma_start(
                out=xt_sb[:, :, nsl],
                in_=xt[:, nsl].rearrange("(cb p) m -> p cb m", p=P),
            )

        # ---------------- finalize ----------------
        # G PSUM -> SBUF upper blocks (split DVE/ACT), mirror via PE matmul
        for r in range(CB):
            for c in range(r, CB):
                bank, off = _blk(_STARTS[r] + c - r)
                if (r + c) % 2 == 0:
                    nc.vector.tensor_copy(
                        out=gsb[:, r, c * P : (c + 1) * P],
                        in_=gtile[bank][:, off : off + P],
                    )
                else:
                    nc.scalar.copy(
                        out=gsb[:, r, c * P : (c + 1) * P],
                        in_=gtile[bank][:, off : off + P],
                    )
        gram_ctx.close()

        with tc.tile_pool(name="tpps", bufs=2, space="PSUM") as tppool:
            for r in range(CB):
                for c in range(r + 1, CB):
                    tp = tppool.tile([P, P], F32, tag="tp")
                    nc.tensor.matmul(
                        tp,
                        lhsT=gsb[:, r, c * P : (c + 1) * P],
                        rhs=ident_h,
                        start=True,
                        stop=True,
                    )
                    nc.vector.tensor_copy(
                        out=gsb[:, c, r * P : (r + 1) * P], in_=tp
                    )

        if dbg:
            nc.scalar.dma_start(out=dbg_g[:, :, :], in_=gsb)

        fs_ctx = ExitStack()
        at_ctx = ExitStack()
        atpool = at_ctx.enter_context(tc.tile_pool(name="atps", bufs=1, space="PSUM"))
        at1 = atpool.tile([HD, 5 * HD], F32, tag="at1", name="at1")
        at2 = atpool.tile([HD, 3 * HD], F32, tag="at2", name="at2")
        a2_ctx = ExitStack()
        a2ps = a2_ctx.enter_context(tc.tile_pool(name="a2ps", bufs=2, space="PSUM"))
        kk_ctx = ExitStack()
        kkpool = kk_ctx.enter_context(tc.tile_pool(name="kkps", bufs=1, space="PSUM"))
        kk1 = kkpool.tile([HD, 5 * HD], F32, tag="kk1", name="kk1")
        kk2 = kkpool.tile([HD, 3 * HD], F32, tag="kk2", name="kk2")
        dscr = fs_ctx.enter_context(tc.tile_pool(name="dscr", bufs=1))

        # ---- phase K: A2k rows + qk/kk grams, software-pipelined ----
        def a2_row(r, j0):
            """A2[:, r, j0:j0+768] = sum_cb G[cb, r]^T @ w2[cb, j0:j0+768]"""
            a2p = a2ps.tile([P, 1024], F32, tag="a2p")
            for cb in range(CB):
                lh = gsb[:, cb, r * P : (r + 1) * P]
                nc.tensor.matmul(
                    a2p[:, 0:512], lhsT=lh, rhs=w2_sb[:, cb, j0 : j0 + 512],
                    start=(cb == 0), stop=(cb == CB - 1),
                    skip_group_check=True,
                )
                nc.tensor.matmul(
                    a2p[:, 512:768], lhsT=lh,
                    rhs=w2_sb[:, cb, j0 + 512 : j0 + 768],
                    start=(cb == 0), stop=(cb == CB - 1),
                    skip_group_check=True,
                )
            if r % 2 == 0:
                nc.vector.tensor_copy(
                    out=a2sb[:, r, j0 : j0 + C], in_=a2p[:, 0:C]
                )
            else:
                nc.scalar.copy(out=a2sb[:, r, j0 : j0 + C], in_=a2p[:, 0:C])

        def grams_k(r):
            for h in range(H):
                b1, b2 = (at1, kk1) if h < 5 else (at2, kk2)
                co = HD * h if h < 5 else HD * (h - 5)
                rhs = a2sb[:, r, C + h * HD : C + (h + 1) * HD]
                nc.tensor.matmul(
                    b1[:, co : co + HD],
                    lhsT=w2_sb[:, r, h * HD : (h + 1) * HD],
                    rhs=rhs,
                    start=(r == 0 and h in (0, 5)), stop=(r == CB - 1),
                    skip_group_check=True,
                )
                nc.tensor.matmul(
                    b2[:, co : co + HD],
                    lhsT=w2_sb[:, r, C + h * HD : C + (h + 1) * HD],
                    rhs=rhs,
                    start=(r == 0 and h in (0, 5)), stop=(r == CB - 1),
                    skip_group_check=True,
                )

        a2_row(0, C)
        for r in range(1, CB):
            a2_row(r, C)
            grams_k(r - 1)
        grams_k(CB - 1)

        # sumsq_k = diag(kk) via identity mask + row reduce
        dk1 = dscr.tile([HD, 5, HD], F32, tag="dk1")
        dk2 = dscr.tile([HD, 3, HD], F32, tag="dk2")
        nc.vector.tensor_tensor(
            out=dk1,
            in0=kk1.rearrange("p (h e) -> p h e", e=HD),
            in1=ident_h[0:HD, None, 0:HD].to_broadcast([HD, 5, HD]),
            op=mybir.AluOpType.mult,
        )
        nc.vector.tensor_tensor(
            out=dk2,
            in0=kk2.rearrange("p (h e) -> p h e", e=HD),
            in1=ident_h[0:HD, None, 0:HD].to_broadcast([HD, 3, HD]),
            op=mybir.AluOpType.mult,
        )
        nc.vector.tensor_reduce(
            out=sumsq_sb[:, H : H + 5], in_=dk1, axis=mybir.AxisListType.X,
            op=mybir.AluOpType.add,
        )
        nc.vector.tensor_reduce(
            out=sumsq_sb[:, H + 5 : 2 * H], in_=dk2, axis=mybir.AxisListType.X,
            op=mybir.AluOpType.add,
        )
        kk_ctx.close()

        # ---- phase Q: A2q rows + qq grams ----
        qq_ctx = ExitStack()
        qqpool = qq_ctx.enter_context(tc.tile_pool(name="qqps", bufs=1, space="PSUM"))
        qq1 = qqpool.tile([HD, 5 * HD], F32, tag="qq1", name="qq1")
        qq2 = qqpool.tile([HD, 3 * HD], F32, tag="qq2", name="qq2")

        def grams_q(r):
            for h in range(H):
                b1 = qq1 if h < 5 else qq2
                co = HD * h if h < 5 else HD * (h - 5)
                nc.tensor.matmul(
                    b1[:, co : co + HD],
                    lhsT=w2_sb[:, r, h * HD : (h + 1) * HD],
                    rhs=a2sb[:, r, h * HD : (h + 1) * HD],
                    start=(r == 0 and h in (0, 5)), stop=(r == CB - 1),
                    skip_group_check=True,
                )

        a2_row(0, 0)
        for r in range(1, CB):
            a2_row(r, 0)
            grams_q(r - 1)
        grams_q(CB - 1)

        dq1 = dscr.tile([HD, 5, HD], F32, tag="dq1")
        dq2 = dscr.tile([HD, 3, HD], F32, tag="dq2")
        nc.vector.tensor_tensor(
            out=dq1,
            in0=qq1.rearrange("p (h e) -> p h e", e=HD),
            in1=ident_h[0:HD, None, 0:HD].to_broadcast([HD, 5, HD]),
            op=mybir.AluOpType.mult,
        )
        nc.vector.tensor_tensor(
            out=dq2,
            in0=qq2.rearrange("p (h e) -> p h e", e=HD),
            in1=ident_h[0:HD, None, 0:HD].to_broadcast([HD, 3, HD]),
            op=mybir.AluOpType.mult,
        )
        nc.vector.tensor_reduce(
            out=sumsq_sb[:, 0:5], in_=dq1, axis=mybir.AxisListType.X,
            op=mybir.AluOpType.add,
        )
        nc.vector.tensor_reduce(
            out=sumsq_sb[:, 5:H], in_=dq2, axis=mybir.AxisListType.X,
            op=mybir.AluOpType.add,
        )
        qq_ctx.close()
        a2_ctx.close()

        if dbg:
            nc.scalar.dma_start(out=dbg_a2[:, :, :], in_=a2sb)
            nc.scalar.dma_start(out=dbg_ss[:, :], in_=sumsq_sb)

        # s = 1/max(sqrt(ss), eps); temperature folded into s_q
        nc.scalar.sqrt(out=s_sb, in_=sumsq_sb)
        nc.vector.tensor_scalar_max(s_sb, s_sb, EPS)
        nc.vector.reciprocal(out=s_sb, in_=s_sb)
        nc.vector.tensor_tensor(
            out=s_sb[:, 0:H], in0=s_sb[:, 0:H], in1=temp_sb,
            op=mybir.AluOpType.mult,
        )

        # combined scale [d,h,e] = s_q[d,h] * s_k[e,h] via ones96^T @ diag
        skrep = dscr.tile([HD, H, HD], F32, tag="skrep")
        with tc.tile_pool(name="skps", bufs=1, space="PSUM") as skpool:
            diag_all = dscr.tile([HD, H, HD], F16, tag="diag_all")
            nc.vector.tensor_tensor(
                out=diag_all,
                in0=ident_h[0:HD, None, 0:HD].to_broadcast([HD, H, HD]),
                in1=s_sb[:, H : 2 * H, None].to_broadcast([HD, H, HD]),
                op=mybir.AluOpType.mult,
            )
            skp = skpool.tile([HD, 1024], F32, tag="skp")
            df = diag_all.rearrange("p h e -> p (h e)")
            nc.tensor.matmul(
                skp[:, 0:512], lhsT=ones96, rhs=df[:, 0:512],
                start=True, stop=True,
            )
            nc.tensor.matmul(
                skp[:, 512:768], lhsT=ones96, rhs=df[:, 512:768],
                start=True, stop=True,
            )
            nc.vector.tensor_copy(
                out=skrep.rearrange("p h e -> p (h e)"), in_=skp[:, 0:768]
            )
            nc.vector.tensor_tensor(
                out=skrep, in0=skrep,
                in1=s_sb[:, 0:H, None].to_broadcast([HD, H, HD]),
                op=mybir.AluOpType.mult,
            )

            # softmax per head-group (no max subtraction: |logit| <= temp)
            # T1_h = attn_h^T @ Wproj_h follows each group on PE
            ga = dscr.tile([HD, H, HD], F32, tag="ga")
            with tc.tile_pool(name="t1ps", bufs=2, space="PSUM") as t1ps:
                for g, (h0, nh) in enumerate(((0, 5), (5, 3))):
                    bank = at1 if g == 0 else at2
                    gag = ga[:, h0 : h0 + nh, :]
                    nc.vector.tensor_copy(
                        out=gag.rearrange("p h e -> p (h e)"), in_=bank
                    )
                    nc.vector.tensor_tensor(
                        out=gag, in0=gag, in1=skrep[:, h0 : h0 + nh, :],
                        op=mybir.AluOpType.mult,
                    )
                    nc.scalar.activation(
                        out=gag, in_=gag,
                        func=mybir.ActivationFunctionType.Exp,
                        bias=0.0, scale=1.0,
                    )
                    rsum = dscr.tile([HD, H], F32, tag=f"rsum{g}")
                    nc.vector.tensor_reduce(
                        out=rsum[:, 0:nh], in_=gag, axis=mybir.AxisListType.X,
                        op=mybir.AluOpType.add,
                    )
                    nc.vector.reciprocal(out=rsum[:, 0:nh], in_=rsum[:, 0:nh])
                    nc.vector.tensor_tensor(
                        out=atsb[:, h0 : h0 + nh, :], in0=gag,
                        in1=rsum[:, 0:nh, None].to_broadcast([HD, nh, HD]),
                        op=mybir.AluOpType.mult,
                    )
                    for h in range(h0, h0 + nh):
                        t1p = t1ps.tile([HD, 1024], F32, tag="t1p")
                        lh = atsb[:, h, :]
                        nc.tensor.matmul(
                            t1p[:, 0:512], lhsT=lh, rhs=wpe_sb[:, h, 0:512],
                            start=True, stop=True,
                        )
                        nc.tensor.matmul(
                            t1p[:, 512:768], lhsT=lh, rhs=wpe_sb[:, h, 512:C],
                            start=True, stop=True,
                        )
                        if h % 2 == 0:
                            nc.vector.tensor_copy(
                                out=t1_sb[:, h, :], in_=t1p[:, 0:C]
                            )
                        else:
                            nc.scalar.copy(out=t1_sb[:, h, :], in_=t1p[:, 0:C])
        at_ctx.close()
        if dbg:
            nc.scalar.dma_start(out=dbg_at[:, :, :], in_=atsb)
            nc.scalar.dma_start(out=dbg_t1[:, :, :], in_=t1_sb)

        # Wbig = sum_h Wv_h @ T1_h
        with tc.tile_pool(name="wbps", bufs=2, space="PSUM") as wbps:
            for m in range(CB):
                wbp = wbps.tile([P, 1024], F32, tag="wbp")
                for h in range(H):
                    lh = wvt_sb[:, h, m * P : (m + 1) * P]
                    nc.tensor.matmul(
                        wbp[:, 0:512], lhsT=lh, rhs=t1_sb[:, h, 0:512],
                        start=(h == 0), stop=(h == H - 1),
                        skip_group_check=True,
                    )
                    nc.tensor.matmul(
                        wbp[:, 512:768], lhsT=lh, rhs=t1_sb[:, h, 512:C],
                        start=(h == 0), stop=(h == H - 1),
                        skip_group_check=True,
                    )
                if m % 2 == 0:
                    nc.vector.tensor_copy(out=wbig_sb[:, m, :], in_=wbp[:, 0:C])
                else:
                    nc.scalar.copy(out=wbig_sb[:, m, :], in_=wbp[:, 0:C])
        if dbg:
            nc.scalar.dma_start(out=dbg_wb[:, :, :], in_=wbig_sb)
        fs_ctx.close()

        # ---------------- pass 2: y^T = Wbig^T @ x^T + b (fp16) -------------
        with tc.tile_pool(name="yps", bufs=8, space="PSUM") as yps, \
             tc.tile_pool(name="ysbp", bufs=4) as ypool:
            for n in range(8):
                nsl = slice(n * 512, (n + 1) * 512)
                for co in range(CB):
                    ypb = yps.tile([P, 512], F32, tag="ypb")
                    for ci in range(CB):
                        nc.tensor.matmul(
                            ypb,
                            lhsT=wbig_sb[:, ci, co * P : (co + 1) * P],
                            rhs=xt_sb[:, ci, nsl],
                            start=(ci == 0),
                            stop=(ci == CB - 1),
                        )
                    ysb = ypool.tile([P, 512], F16, tag="ysb")
                    nc.vector.tensor_tensor(
                        out=ysb,
                        in0=ypb,
                        in1=bias_sb[:, co : co + 1].to_broadcast([P, 512]),
                        op=mybir.AluOpType.add,
                    )
                    nc.sync.dma_start(
                        out=yt[co * P : (co + 1) * P, nsl], in_=ysb
                    )

    nc.compile()
    return nc


def prep_inputs(x, Wqkv, temperature, Wproj, bproj):
    B = x.shape[0]
    wqkv = np.asarray(Wqkv, dtype=np.float32)
    w2 = np.ascontiguousarray(wqkv[:, :C2].astype(np.float16))
    wvt = np.ascontiguousarray(
        wqkv[:, C2:].T.reshape(H, HD, C).transpose(1, 0, 2).astype(np.float16)
    )
    wpe = np.ascontiguousarray(
        np.asarray(Wproj, dtype=np.float32)
        .reshape(H, HD, C)
        .transpose(1, 0, 2)
        .astype(np.float16)
    )
    temp = np.ascontiguousarray(np.asarray(temperature, dtype=np.float32).reshape(H))
    biasE = np.ascontiguousarray(
        np.asarray(bproj, dtype=np.float32).reshape(CB, P).T
    )
    xf = np.asarray(x, dtype=np.float32)
    in_maps = [
        {
            "x": np.ascontiguousarray(xf[b].astype(np.float16)),
            "xt": np.ascontiguousarray(xf[b].T.astype(np.float16)),
            "w2": w2,
            "wvt": wvt,
            "wpe": wpe,
            "temp": temp,
            "biasE": biasE,
        }
        for b in range(B)
    ]
    return in_maps


def kernel(x, Wqkv, temperature, Wproj, bproj):
    from concourse.bass_utils import run_bass_kernel_spmd

    B = x.shape[0]
    key = "nc"
    if key not in _CACHE:
        _CACHE[key] = _build()
    nc = _CACHE[key]

    in_maps = prep_inputs(x, Wqkv, temperature, Wproj, bproj)
    res = run_bass_kernel_spmd(nc, in_maps, core_ids=list(range(B)))
    out = np.stack(
        [res.results[b]["yt"].T.astype(np.float32) for b in range(B)], axis=0
    )
    return np.ascontiguousarray(out)


if __name__ == "__main__":
    rng = np.random.default_rng(0)
    inputs = {
        "x": rng.standard_normal((8, N, C), dtype=np.float32),
        "Wqkv": (rng.standard_normal((C, NC3)) / np.sqrt(C)).astype(np.float32),
        "temperature": np.ones((H, 1, 1), dtype=np.float32),
        "Wproj": (rng.standard_normal((C, C)) / np.sqrt(C)).astype(np.float32),
        "bproj": (rng.standard_normal(C) * 0.01).astype(np.float32),
    }
    out = kernel(**inputs)
    print(out.shape, out.dtype)


# revision 7
# speedup vs baseline: 1.3126x; 1.0658x over previous
"""ChannelAttention Trainium2 Bass kernel (v3).

Data-parallel over batch: 8 batches -> 8 NeuronCores, zero communication.

Key algebra: q,k are never materialized.  With G = x^T x  [C, C]:
  gram_qk_h = Wq_h^T G Wk_h          (attention logits numerator)
  sumsq_q   = diag(Wq_h^T G Wq_h)    (token-dim L2 norms of q)
  sumsq_k   = diag(Wk_h^T G Wk_h)
Pass 1 accumulates G (upper triangle) from token chunks streamed over
BOTH HWDGE rings (even chunks sync, odd chunks scalar); weights and the
x^T prefetch queue behind them.  Finalize (fp16 operands, fp32 PSUM):
  phase K (rows descending, so row 5 needs no mirrored G blocks):
      A2k = G @ Wk, grams Wq_h^T A2k_h (qk) and Wk_h^T A2k_h (kk)
  phase Q: A2q = G @ Wq, grams Wq_h^T A2q_h (qq); the s_k/softmax-scale
      DVE chain runs concurrently with phase Q's PE work
  softmax, T1_h = attn_h^T Wproj_h, Wbig = sum_h Wv_h @ T1_h   [C, C]
Pass 2: y^T = Wbig^T @ x^T + b from SBUF-resident x^T; output rides
both rings.  Host transposes the output.

Gram lhsT slices are padded to 128 columns (extra output partitions are
never read) so LDWEIGHTS gets fast-weight-load.  fp16 everywhere: same
PE speed as bf16, 8x finer mantissa; |x|<6, |G|<5k, |A2|<1k << 65504.
"""

import sys

if "/opt/trn_rl_repo" not in sys.path:
    sys.path.insert(0, "/opt/trn_rl_repo")

import numpy as np

N, C, H, HD = 4096, 768, 8, 96
C2 = 2 * C
NC3 = 3 * C
EPS = 1e-12
P = 128
CB = C // P            # 6 channel blocks
NCH2 = N // 256        # 16 double token chunks
W2PAD = C2 + 32        # fp16 w2 padded so 128-wide lhsT slices stay in bounds

# upper-triangle block packing: block (r, c), r <= c, index b -> bank b//4,
# column offset (b%4)*128 inside PSUM tiles of [128, 512]
_STARTS = [0, 6, 11, 15, 18, 20]
# per row: list of (bank, offset, c0, ncols) matmul runs covering cols c0..
_G_RUNS = {
    0: [(0, 0, 0, 512), (1, 0, 4, 256)],
    1: [(1, 256, 1, 256), (2, 0, 3, 384)],
    2: [(2, 384, 2, 128), (3, 0, 3, 384)],
    3: [(3, 384, 3, 128), (4, 0, 4, 256)],
    4: [(4, 256, 4, 256)],
    5: [(5, 0, 5, 128)],
}

_CACHE = {}


def _blk(b):
    return b // 4, (b % 4) * P


def _build(dbg=False):
    import concourse.bacc as bacc
    import concourse.tile as tile
    import concourse.mybir as mybir
    from concourse.masks import make_identity
    from contextlib import ExitStack

    F32 = mybir.dt.float32
    F16 = mybir.dt.float16

    nc = bacc.Bacc("TRN2", target_bir_lowering=False, debug=False, num_devices=8)
    x = nc.dram_tensor("x", [N, C], F16, kind="ExternalInput")
    xt = nc.dram_tensor("xt", [C, N], F16, kind="ExternalInput")
    w2 = nc.dram_tensor("w2", [C, C2], F16, kind="ExternalInput")
    wvt = nc.dram_tensor("wvt", [HD, H, C], F16, kind="ExternalInput")
    wpe = nc.dram_tensor("wpe", [HD, H, C], F16, kind="ExternalInput")
    temp = nc.dram_tensor("temp", [H], F32, kind="ExternalInput")
    biasE = nc.dram_tensor("biasE", [P, CB], F32, kind="ExternalInput")
    yt = nc.dram_tensor("yt", [C, N], F16, kind="ExternalOutput")
    if dbg:
        dbg_g = nc.dram_tensor("dbg_g", [P, CB, C], F16, kind="ExternalOutput")
        dbg_a2 = nc.dram_tensor("dbg_a2", [P, CB, C2], F16, kind="ExternalOutput")
        dbg_ss = nc.dram_tensor("dbg_ss", [HD, 2 * H], F32, kind="ExternalOutput")
        dbg_at = nc.dram_tensor("dbg_at", [HD, H, HD], F16, kind="ExternalOutput")
        dbg_wb = nc.dram_tensor("dbg_wb", [P, CB, C], F16, kind="ExternalOutput")

    with tile.TileContext(nc) as tc, ExitStack() as ctx:
        singles = ctx.enter_context(tc.tile_pool(name="singles", bufs=1))
        ident_f = singles.tile([P, P], F32)
        ident_h = singles.tile([P, P], F16)
        ones_h = singles.tile([HD, P], F16)
        temp_sb = singles.tile([HD, H], F32)
        bias_sb = singles.tile([P, CB], F32)
        s_sb = singles.tile([HD, 2 * H], F32)
        sumsq_sb = singles.tile([HD, 2 * H], F32)
        atsb = singles.tile([HD, H, P], F16)
        xt_sb = singles.tile([P, CB, N], F16)
        w2_sb = singles.tile([P, CB, W2PAD], F16)
        wvt_sb = singles.tile([HD, H, C], F16)
        wpe_sb = singles.tile([HD, H, C], F16)
        gsb = singles.tile([P, CB, C], F16)
        a2sb = singles.tile([P, CB, C2], F16)
        t1_sb = singles.tile([HD, H, C], F16)
        wbig_sb = singles.tile([P, CB, C], F16)

        make_identity(nc, ident_f)
        nc.vector.tensor_copy(out=ident_h, in_=ident_f)
        nc.vector.memset(ones_h, 1.0)
        nc.vector.memset(atsb, 0.0)
        nc.vector.memset(w2_sb[:, :, C2:W2PAD], 0.0)
        nc.scalar.dma_start(out=temp_sb, in_=temp[None, :].to_broadcast([HD, H]))
        nc.scalar.dma_start(out=bias_sb, in_=biasE[:, :])

        # ---------------- pass 1: G = x^T x (upper triangle) ----------------
        # x double-chunks alternate sync/scalar rings; weights and the x^T
        # prefetch are emitted after the loop so they queue behind the x
        # stream on their rings.
        gram_ctx = ExitStack()
        gram_pool = gram_ctx.enter_context(
            tc.tile_pool(name="gps", bufs=1, space="PSUM")
        )
        gtile = [
            gram_pool.tile([P, 512], F32, tag=f"g{i}", name=f"g{i}")
            for i in range(6)
        ]

        with tc.tile_pool(name="p1", bufs=12) as p1pool:
            for i in range(NCH2):
                xc = p1pool.tile([P, 2, C], F16, tag="xc")
                ring = nc.sync if i % 2 == 0 else nc.scalar
                ring.dma_start(
                    out=xc,
                    in_=x[i * 256 : (i + 1) * 256, :].rearrange(
                        "(two p) c -> p two c", p=P
                    ),
                )
                for t in range(2):
                    for r in range(CB):
                        lh = xc[:, t, r * P : (r + 1) * P]
                        for (bank, off, c0, ncols) in _G_RUNS[r]:
                            nc.tensor.matmul(
                                gtile[bank][:, off : off + ncols],
                                lhsT=lh,
                                rhs=xc[:, t, c0 * P : c0 * P + ncols],
                                start=(i == 0 and t == 0 and off == 0),
                                stop=(i == NCH2 - 1 and t == 1),
                                skip_group_check=True,
                            )

        nc.scalar.dma_start(
            out=w2_sb[:, :, 0:C2], in_=w2.rearrange("(cb p) j -> p cb j", p=P)
        )
        nc.scalar.dma_start(out=wvt_sb, in_=wvt[:, :, :])
        nc.scalar.dma_start(out=wpe_sb, in_=wpe[:, :, :])
        for n in range(8):
            nsl = slice(n * 512, (n + 1) * 512)
            nc.sync.dma_start(
                out=xt_sb[:, :, nsl],
                in_=xt[:, nsl].rearrange("(cb p) m -> p cb m", p=P),
            )

        # ---------------- finalize ----------------
        # G PSUM -> SBUF upper blocks, column-descending so A2 row 5 (which
        # needs no mirrored blocks) can start immediately; mirror transposes
        # run on PE underneath A2 row 5.
        for c in range(CB - 1, -1, -1):
            for r in range(0, c + 1):
                bank, off = _blk(_STARTS[r] + c - r)
                if (r + c) % 2 == 0:
                    nc.vector.tensor_copy(
                        out=gsb[:, r, c * P : (c + 1) * P],
                        in_=gtile[bank][:, off : off + P],
                    )
                else:
                    nc.scalar.copy(
                        out=gsb[:, r, c * P : (c + 1) * P],
                        in_=gtile[bank][:, off : off + P],
                    )
        gram_ctx.close()

        fs_ctx = ExitStack()
        at_ctx = ExitStack()
        atpool = at_ctx.enter_context(tc.tile_pool(name="atps", bufs=1, space="PSUM"))
        at1 = atpool.tile([P, 5 * HD], F32, tag="at1", name="at1")
        at2 = atpool.tile([P, 3 * HD], F32, tag="at2", name="at2")
        a2_ctx = ExitStack()
        a2ps = a2_ctx.enter_context(tc.tile_pool(name="a2ps", bufs=2, space="PSUM"))
        dscr = fs_ctx.enter_context(tc.tile_pool(name="dscr", bufs=1))

        def a2_row(r, j0):
            """A2[:, r, j0:j0+768] = sum_cb G[cb, r]^T @ w2[cb, j0:j0+768]"""
            a2p = a2ps.tile([P, 1024], F32, tag="a2p")
            for cb in range(CB):
                lh = gsb[:, cb, r * P : (r + 1) * P]
                nc.tensor.matmul(
                    a2p[:, 0:512], lhsT=lh, rhs=w2_sb[:, cb, j0 : j0 + 512],
                    start=(cb == 0), stop=(cb == CB - 1),
                    skip_group_check=True,
                )
                nc.tensor.matmul(
                    a2p[:, 512:768], lhsT=lh,
                    rhs=w2_sb[:, cb, j0 + 512 : j0 + 768],
                    start=(cb == 0), stop=(cb == CB - 1),
                    skip_group_check=True,
                )
            if r % 2 == 0:
                nc.vector.tensor_copy(
                    out=a2sb[:, r, j0 : j0 + C], in_=a2p[:, 0:C]
                )
            else:
                nc.scalar.copy(out=a2sb[:, r, j0 : j0 + C], in_=a2p[:, 0:C])

        # phase K row 5 first: all its G blocks are direct upper copies
        a2_row(5, C)

        # mirror the lower triangle of G via PE transposes (under A2 row 5)
        with tc.tile_pool(name="tpps", bufs=2, space="PSUM") as tppool:
            for r in range(CB - 2, -1, -1):
                for c in range(r + 1, CB):
                    tp = tppool.tile([P, P], F32, tag="tp")
                    nc.tensor.matmul(
                        tp,
                        lhsT=gsb[:, r, c * P : (c + 1) * P],
                        rhs=ident_h,
                        start=True,
                        stop=True,
                    )
                    nc.vector.tensor_copy(
                        out=gsb[:, c, r * P : (r + 1) * P], in_=tp
                    )

        kk_ctx = ExitStack()
        kkpool = kk_ctx.enter_context(tc.tile_pool(name="kkps", bufs=1, space="PSUM"))
        kk1 = kkpool.tile([P, 5 * HD], F32, tag="kk1", name="kk1")
        kk2 = kkpool.tile([P, 3 * HD], F32, tag="kk2", name="kk2")

        def grams_k(r, first, last):
            for h in range(H):
                b1, b2 = (at1, kk1) if h < 5 else (at2, kk2)
                co = HD * h if h < 5 else HD * (h - 5)
                rhs = a2sb[:, r, C + h * HD : C + (h + 1) * HD]
                nc.tensor.matmul(
                    b1[:, co : co + HD],
                    lhsT=w2_sb[:, r, h * HD : h * HD + P],
                    rhs=rhs,
                    start=(first and h in (0, 5)), stop=last,
                    skip_group_check=True,
                )
                nc.tensor.matmul(
                    b2[:, co : co + HD],
                    lhsT=w2_sb[:, r, C + h * HD : C + h * HD + P],
                    rhs=rhs,
                    start=(first and h in (0, 5)), stop=last,
                    skip_group_check=True,
                )

        rows = [4, 3, 2, 1, 0]
        a2_row(4, C)
        grams_k(5, True, False)
        for idx, r in enumerate(rows[1:]):
            a2_row(r, C)
            grams_k(rows[idx], False, False)
        grams_k(0, False, True)

        def diag_extract(b1, b2, out5, out3):
            d1 = dscr.tile([HD, 5, HD], F32, tag="dg1")
            d2 = dscr.tile([HD, 3, HD], F32, tag="dg2")
            nc.vector.tensor_tensor(
                out=d1,
                in0=b1[0:HD, :].rearrange("p (h e) -> p h e", e=HD),
                in1=ident_h[0:HD, None, 0:HD].to_broadcast([HD, 5, HD]),
                op=mybir.AluOpType.mult,
            )
            nc.vector.tensor_tensor(
                out=d2,
                in0=b2[0:HD, :].rearrange("p (h e) -> p h e", e=HD),
                in1=ident_h[0:HD, None, 0:HD].to_broadcast([HD, 3, HD]),
                op=mybir.AluOpType.mult,
            )
            nc.vector.tensor_reduce(
                out=out5, in_=d1, axis=mybir.AxisListType.X,
                op=mybir.AluOpType.add,
            )
            nc.vector.tensor_reduce(
                out=out3, in_=d2, axis=mybir.AxisListType.X,
                op=mybir.AluOpType.add,
            )

        # ---- sumsq_k extraction + s_k chain (overlaps phase Q's PE work) --
        diag_extract(kk1, kk2, sumsq_sb[:, H : H + 5], sumsq_sb[:, H + 5 : 2 * H])
        kk_ctx.close()
        nc.scalar.sqrt(out=s_sb[:, H : 2 * H], in_=sumsq_sb[:, H : 2 * H])
        nc.vector.reciprocal(out=s_sb[:, H : 2 * H], in_=s_sb[:, H : 2 * H])
        diag_all = dscr.tile([HD, H, HD], F16, tag="diag_all")
        nc.vector.tensor_tensor(
            out=diag_all,
            in0=ident_h[0:HD, None, 0:HD].to_broadcast([HD, H, HD]),
            in1=s_sb[:, H : 2 * H, None].to_broadcast([HD, H, HD]),
            op=mybir.AluOpType.mult,
        )

        # ---- phase Q (rows descending) ----
        qq_ctx = ExitStack()
        qqpool = qq_ctx.enter_context(tc.tile_pool(name="qqps", bufs=1, space="PSUM"))
        qq1 = qqpool.tile([P, 5 * HD], F32, tag="qq1", name="qq1")
        qq2 = qqpool.tile([P, 3 * HD], F32, tag="qq2", name="qq2")

        def grams_q(r, first, last):
            for h in range(H):
                b1 = qq1 if h < 5 else qq2
                co = HD * h if h < 5 else HD * (h - 5)
                nc.tensor.matmul(
                    b1[:, co : co + HD],
                    lhsT=w2_sb[:, r, h * HD : h * HD + P],
                    rhs=a2sb[:, r, h * HD : (h + 1) * HD],
                    start=(first and h in (0, 5)), stop=last,
                    skip_group_check=True,
                )

        a2_row(5, 0)
        a2_row(4, 0)
        grams_q(5, True, False)
        for idx, r in enumerate(rows[1:]):
            a2_row(r, 0)
            grams_q(rows[idx], False, False)
        grams_q(0, False, True)

        diag_extract(qq1, qq2, sumsq_sb[:, 0:5], sumsq_sb[:, 5:H])
        qq_ctx.close()
        a2_ctx.close()
        nc.scalar.sqrt(out=s_sb[:, 0:H], in_=sumsq_sb[:, 0:H])
        nc.vector.reciprocal(out=s_sb[:, 0:H], in_=s_sb[:, 0:H])
        nc.vector.tensor_tensor(
            out=s_sb[:, 0:H], in0=s_sb[:, 0:H], in1=temp_sb,
            op=mybir.AluOpType.mult,
        )

        if dbg:
            nc.scalar.dma_start(out=dbg_g[:, :, :], in_=gsb)
            nc.scalar.dma_start(out=dbg_a2[:, :, :], in_=a2sb)
            nc.scalar.dma_start(out=dbg_ss[:, :], in_=sumsq_sb)

        # combined scale [d,h,e] = s_q[d,h] * s_k[e,h] via ones^T @ diag_all
        skrep = dscr.tile([HD, H, HD], F32, tag="skrep")
        with tc.tile_pool(name="skps", bufs=1, space="PSUM") as skpool:
            skp = skpool.tile([P, 1024], F32, tag="skp")
            df = diag_all.rearrange("p h e -> p (h e)")
            nc.tensor.matmul(
                skp[:, 0:512], lhsT=ones_h, rhs=df[:, 0:512],
                start=True, stop=True,
            )
            nc.tensor.matmul(
                skp[:, 512:768], lhsT=ones_h, rhs=df[:, 512:768],
                start=True, stop=True,
            )
            nc.vector.tensor_copy(
                out=skrep.rearrange("p h e -> p (h e)"), in_=skp[0:HD, 0:768]
            )
            nc.vector.tensor_tensor(
                out=skrep, in0=skrep,
                in1=s_sb[:, 0:H, None].to_broadcast([HD, H, HD]),
                op=mybir.AluOpType.mult,
            )

            # softmax per head-group (no max subtraction: |logit| <= temp)
            # T1_h = attn_h^T @ Wproj_h follows each group on PE
            ga = dscr.tile([HD, H, HD], F32, tag="ga")
            with tc.tile_pool(name="t1ps", bufs=2, space="PSUM") as t1ps:
                for g, (h0, nh) in enumerate(((0, 5), (5, 3))):
                    bank = at1 if g == 0 else at2
                    gag = ga[:, h0 : h0 + nh, :]
                    nc.vector.tensor_tensor(
                        out=gag,
                        in0=bank[0:HD, :].rearrange("p (h e) -> p h e", e=HD),
                        in1=skrep[:, h0 : h0 + nh, :],
                        op=mybir.AluOpType.mult,
                    )
                    nc.scalar.activation(
                        out=gag, in_=gag,
                        func=mybir.ActivationFunctionType.Exp,
                        bias=0.0, scale=1.0,
                    )
                    rsum = dscr.tile([HD, H], F32, tag=f"rsum{g}")
                    nc.vector.tensor_reduce(
                        out=rsum[:, 0:nh], in_=gag, axis=mybir.AxisListType.X,
                        op=mybir.AluOpType.add,
                    )
                    nc.vector.reciprocal(out=rsum[:, 0:nh], in_=rsum[:, 0:nh])
                    nc.vector.tensor_tensor(
                        out=atsb[:, h0 : h0 + nh, 0:HD], in0=gag,
                        in1=rsum[:, 0:nh, None].to_broadcast([HD, nh, HD]),
                        op=mybir.AluOpType.mult,
                    )
                    for h in range(h0, h0 + nh):
                        t1p = t1ps.tile([P, 1024], F32, tag="t1p")
                        lh = atsb[:, h, :]
                        nc.tensor.matmul(
                            t1p[:, 0:512], lhsT=lh, rhs=wpe_sb[:, h, 0:512],
                            start=True, stop=True,
                        )
                        nc.tensor.matmul(
                            t1p[:, 512:768], lhsT=lh, rhs=wpe_sb[:, h, 512:C],
                            start=True, stop=True,
                        )
                        if h % 2 == 0:
                            nc.vector.tensor_copy(
                                out=t1_sb[:, h, :], in_=t1p[0:HD, 0:C]
                            )
                        else:
                            nc.scalar.copy(
                                out=t1_sb[:, h, :], in_=t1p[0:HD, 0:C]
                            )
        at_ctx.close()
        if dbg:
            nc.scalar.dma_start(out=dbg_at[:, :, :], in_=atsb[:, :, 0:HD])

        # Wbig = sum_h Wv_h @ T1_h
        with tc.tile_pool(name="wbps", bufs=2, space="PSUM") as wbps:
            for m in range(CB):
                wbp = wbps.tile([P, 1024], F32, tag="wbp")
                for h in range(H):
                    lh = wvt_sb[:, h, m * P : (m + 1) * P]
                    nc.tensor.matmul(
                        wbp[:, 0:512], lhsT=lh, rhs=t1_sb[:, h, 0:512],
                        start=(h == 0), stop=(h == H - 1),
                        skip_group_check=True,
                    )
                    nc.tensor.matmul(
                        wbp[:, 512:768], lhsT=lh, rhs=t1_sb[:, h, 512:C],
                        start=(h == 0), stop=(h == H - 1),
                        skip_group_check=True,
                    )
                if m % 2 == 0:
                    nc.vector.tensor_copy(out=wbig_sb[:, m, :], in_=wbp[:, 0:C])
                else:
                    nc.scalar.copy(out=wbig_sb[:, m, :], in_=wbp[:, 0:C])
        if dbg:
            nc.scalar.dma_start(out=dbg_wb[:, :, :], in_=wbig_sb)
        fs_ctx.close()

        # ---------------- pass 2: y^T = Wbig^T @ x^T + b (fp16) -------------
        with tc.tile_pool(name="yps", bufs=8, space="PSUM") as yps, \
             tc.tile_pool(name="ysbp", bufs=4) as ypool:
            for n in range(8):
                nsl = slice(n * 512, (n + 1) * 512)
                for co in range(CB):
                    ypb = yps.tile([P, 512], F32, tag="ypb")
                    for ci in range(CB):
                        nc.tensor.matmul(
                            ypb,
                            lhsT=wbig_sb[:, ci, co * P : (co + 1) * P],
                            rhs=xt_sb[:, ci, nsl],
                            start=(ci == 0),
                            stop=(ci == CB - 1),
                        )
                    ysb = ypool.tile([P, 512], F16, tag="ysb")
                    nc.vector.tensor_tensor(
                        out=ysb,
                        in0=ypb,
                        in1=bias_sb[:, co : co + 1].to_broadcast([P, 512]),
                        op=mybir.AluOpType.add,
                    )
                    ring = nc.sync if co % 2 == 0 else nc.scalar
                    ring.dma_start(out=yt[co * P : (co + 1) * P, nsl], in_=ysb)

    nc.compile()
    return nc


def prep_inputs(x, Wqkv, temperature, Wproj, bproj):
    B = x.shape[0]
    wqkv = np.asarray(Wqkv, dtype=np.float32)
    w2 = np.ascontiguousarray(wqkv[:, :C2].astype(np.float16))
    wvt = np.ascontiguousarray(
        wqkv[:, C2:].T.reshape(H, HD, C).transpose(1, 0, 2).astype(np.float16)
    )
    wpe = np.ascontiguousarray(
        np.asarray(Wproj, dtype=np.float32)
        .reshape(H, HD, C)
        .transpose(1, 0, 2)
        .astype(np.float16)
    )
    temp = np.ascontiguousarray(np.asarray(temperature, dtype=np.float32).reshape(H))
    biasE = np.ascontiguousarray(
        np.asarray(bproj, dtype=np.float32).reshape(CB, P).T
    )
    xf = np.asarray(x, dtype=np.float32)
    in_maps = [
        {
            "x": np.ascontiguousarray(xf[b].astype(np.float16)),
            "xt": np.ascontiguousarray(xf[b].T.astype(np.float16)),
            "w2": w2,
            "wvt": wvt,
            "wpe": wpe,
            "temp": temp,
            "biasE": biasE,
        }
        for b in range(B)
    ]
    return in_maps


def kernel(x, Wqkv, temperature, Wproj, bproj):
    from concourse.bass_utils import run_bass_kernel_spmd

    B = x.shape[0]
    key = "nc"
    if key not in _CACHE:
        _CACHE[key] = _build()
    nc = _CACHE[key]

    in_maps = prep_inputs(x, Wqkv, temperature, Wproj, bproj)
    res = run_bass_kernel_spmd(nc, in_maps, core_ids=list(range(B)))
    out = np.stack(
        [res.results[b]["yt"].T.astype(np.float32) for b in range(B)], axis=0
    )
    return np.ascontiguousarray(out)


if __name__ == "__main__":
    rng = np.random.default_rng(0)
    inputs = {
        "x": rng.standard_normal((8, N, C), dtype=np.float32),
        "Wqkv": (rng.standard_normal((C, NC3)) / np.sqrt(C)).astype(np.float32),
        "temperature": np.ones((H, 1, 1), dtype=np.float32),
        "Wproj": (rng.standard_normal((C, C)) / np.sqrt(C)).astype(np.float32),
        "bproj": (rng.standard_normal(C) * 0.01).astype(np.float32),
    }
    out = kernel(**inputs)
    print(out.shape, out.dtype)
